# revision 13
# baseline (speedup 1.0000x reference)
"""GCN (2x GraphConv + BatchNorm + Linear) forward on 8 Trainium2 NeuronCores.

Sharding: data-parallel over the batch axis -- each core owns one whole graph,
so the gather/segment-sum stays core-local.  The big lin_W contraction is
reformulated per-channel:

  out[b,c] = sum_f a[f] * P[b,c,f] + sum_f d[f] * S[c,f] + lin_b[c]

where P[b,c,f] = sum_n h2[b,n,f] * lin_W[c, n*F+f], S[c,f] = sum_n lin_W[c,n*F+f],
and (a, d) are the BatchNorm affine coefficients derived from global mean/var.
Each core computes its graph's h2, BN partial sums (sum h2, sum h2^2), P and S
on device; the host combines the tiny per-core results.

Device algorithm per core (all fp32):
  prep : load x, scale rows by rsqrt(deg_out), store xs to HBM
  layer: for each 128-node dst slice: dma_gather xs[src] rows (edges sorted by
         dst slice, padded per-slice), build one-hot matrices from local dst
         indices on DVE, scatter via PE matmul (gathered^T @ onehot -> agg^T),
         conv matmul with W, PE transpose, scale by rsqrt(deg_in), +bias,
         (relu * rsqrt(deg_out) for layer 1) -> h1 to HBM / h2 stays in SBUF
  tail : stream lin_W, accumulate P, S and BN sums via ones-matmuls in PSUM.
"""

import math
import os
from contextlib import ExitStack

import numpy as np

import concourse.bass as bass
import concourse.tile as tile
from concourse import bacc, mybir
from concourse.bass_utils import run_bass_kernel_spmd

F32 = mybir.dt.float32
I16 = mybir.dt.int16
AF = mybir.ActivationFunctionType
ALU = mybir.AluOpType

BN_EPS = 1e-5

# Set to False to fall back to per-block one-hot builds (no stride-0 APs).
USE_BIG_OH = True
USE_BIG_PROD = True
USE_BARRIERS = True
INTERLEAVE_TAIL = False


# ---------------------------------------------------------------- host prep

def _prep_graph(src, dst, n_nodes, eps):
    """Sort edges by (dst slice, src), pad each slice to `eps` edges.

    Returns (idx16, dstloc, rs_out, rs_in):
      idx16  [128, npad//16] int16  gather indices, edge i at [i%16, i//16]
      dstloc [128, npad//128] f32   local dst (0..127) per edge, 128.0 = pad
      rs_out [128, nslice] f32      rsqrt(max(out_deg,1)),  n = s*128 + p
      rs_in  [128, nslice] f32      rsqrt(max(in_deg,1))
    """
    nslice = n_nodes // 128
    deg_out = np.bincount(src, minlength=n_nodes).astype(np.float32)
    deg_in = np.bincount(dst, minlength=n_nodes).astype(np.float32)
    rs_out = (1.0 / np.sqrt(np.maximum(deg_out, 1.0))).astype(np.float32)
    rs_in = (1.0 / np.sqrt(np.maximum(deg_in, 1.0))).astype(np.float32)
    rs_out_t = rs_out.reshape(nslice, 128).T.copy()
    rs_in_t = rs_in.reshape(nslice, 128).T.copy()

    sl = dst >> 7
    order = np.lexsort((src, sl))
    src_s = src[order].astype(np.int64)
    dst_s = dst[order].astype(np.int64)
    sl_s = sl[order]
    counts = np.bincount(sl_s, minlength=nslice)
    assert counts.max() <= eps, (counts.max(), eps)

    npad = nslice * eps
    src_pad = np.zeros(npad, np.int16)
    dstloc_pad = np.full(npad, 128.0, np.float32)
    starts = np.zeros(nslice + 1, np.int64)
    np.cumsum(counts, out=starts[1:])
    # position of edge k (sorted) inside padded layout
    within = np.arange(len(src_s)) - starts[sl_s]
    pos = sl_s * eps + within
    src_pad[pos] = src_s.astype(np.int16)
    dstloc_pad[pos] = (dst_s & 127).astype(np.float32)

    idx16 = np.tile(src_pad.reshape(-1, 16).T, (8, 1))  # replicated across Q7 cores
    dstloc = dstloc_pad.reshape(-1, 128).T.copy()  # [128, npad//128]
    return idx16, dstloc, rs_out_t, rs_in_t


# ---------------------------------------------------------------- device build

def _build_program(n_nodes, feat, n_edges_pad_per_slice, n_cls, n_cores, gsl):
    """Build the Bass program. Returns (nc, out_name)."""
    NS = n_nodes // 128          # dst slices == node chunks
    F = feat
    EPS = n_edges_pad_per_slice  # padded edges per slice, multiple of 128
    NBLK = EPS // 128            # 128-edge blocks per slice
    NPAD = NS * EPS
    CF = n_cls * F
    GSL = gsl                    # slices per dma_gather call
    assert NS % GSL == 0
    NG = NS // GSL
    IDXW = GSL * EPS // 16       # idx columns per gather call

    nc = bacc.Bacc(
        "TRN2", target_bir_lowering=False, debug=False, num_devices=n_cores
    )

    x_d = nc.dram_tensor("x", [n_nodes, F], F32, kind="ExternalInput")
    idx_d = nc.dram_tensor("idx", [128, NPAD // 16], I16, kind="ExternalInput")
    dstloc_d = nc.dram_tensor("dstloc", [128, NPAD // 128], F32, kind="ExternalInput")
    rs_out_d = nc.dram_tensor("rs_out", [128, NS], F32, kind="ExternalInput")
    rs_in_d = nc.dram_tensor("rs_in", [128, NS], F32, kind="ExternalInput")
    iota_d = nc.dram_tensor("iota", [128, 128], F32, kind="ExternalInput")
    ident_d = nc.dram_tensor("ident", [F, F], F32, kind="ExternalInput")
    w1_d = nc.dram_tensor("W1", [F, F], F32, kind="ExternalInput")
    w2_d = nc.dram_tensor("W2", [F, F], F32, kind="ExternalInput")
    b1_d = nc.dram_tensor("b1b", [128, F], F32, kind="ExternalInput")
    b2_d = nc.dram_tensor("b2b", [128, F], F32, kind="ExternalInput")
    ones_d = nc.dram_tensor("ones", [128, 1], F32, kind="ExternalInput")
    lw_d = nc.dram_tensor("lin_W", [n_cls, n_nodes * F], F32, kind="ExternalInput")

    out_d = nc.dram_tensor("out", [1, 2 * CF + 128], F32, kind="ExternalOutput")

    debug = bool(os.environ.get("GCN_DEBUG"))
    kind_i = "ExternalOutput" if debug else "Internal"
    xs_d = nc.dram_tensor("xs_i", [n_nodes, F], F32, kind=kind_i)
    h1_d = nc.dram_tensor("h1_i", [n_nodes, F], F32, kind=kind_i)
    h2_d = (nc.dram_tensor("h2_i", [128, NS * F], F32, kind="ExternalOutput")
            if debug else None)

    with tile.TileContext(nc) as tc, ExitStack() as ctx:
        cpool = ctx.enter_context(tc.tile_pool(name="const", bufs=1))
        iota_sb = cpool.tile([128, 128], F32, tag="iota")
        ident_sb = cpool.tile([F, F], F32, tag="ident")
        w1_sb = cpool.tile([F, F], F32, tag="w1")
        w2_sb = cpool.tile([F, F], F32, tag="w2")
        b1_sb = cpool.tile([128, F], F32, tag="b1")
        b2_sb = cpool.tile([128, F], F32, tag="b2")
        ones_sb = cpool.tile([128, 1], F32, tag="ones")
        rs_out_sb = cpool.tile([128, NS], F32, tag="rso")
        rs_in_sb = cpool.tile([128, NS], F32, tag="rsi")
        dstloc_sb = cpool.tile([128, NPAD // 128], F32, tag="dstloc")
        idx_sb = cpool.tile([128, NPAD // 16], I16, tag="idx")
        h2_sb = cpool.tile([128, NS * F], F32, tag="h2")

        for t, d in [
            (iota_sb, iota_d), (ident_sb, ident_d), (w1_sb, w1_d),
            (w2_sb, w2_d), (b1_sb, b1_d), (b2_sb, b2_d), (ones_sb, ones_d),
            (rs_out_sb, rs_out_d), (rs_in_sb, rs_in_d),
            (dstloc_sb, dstloc_d), (idx_sb, idx_d),
        ]:
            nc.sync.dma_start(t[:], d.ap())

        # ---- prep: xs = x * rs_out (per node), store to HBM
        with tc.tile_pool(name="prep", bufs=1) as ppool:
            x_sb = ppool.tile([128, NS * F], F32, tag="xsb")
            nc.sync.dma_start(
                x_sb[:].rearrange("p (s f) -> p s f", f=F),
                x_d.ap().rearrange("(s p) f -> p s f", p=128),
            )
            for s in range(NS):
                nc.vector.tensor_scalar(
                    x_sb[:, s * F:(s + 1) * F], x_sb[:, s * F:(s + 1) * F],
                    rs_out_sb[:, s:s + 1], None, op0=ALU.mult,
                )
            nc.sync.dma_start(
                xs_d.ap().rearrange("(s p) f -> p s f", p=128),
                x_sb[:].rearrange("p (s f) -> p s f", f=F),
            )

        if USE_BARRIERS:
            tc.strict_bb_all_engine_barrier()

        # ---- tail pools (P, S, BN accumulation), usable inside layer 2
        lwpool = ctx.enter_context(tc.tile_pool(name="lw", bufs=3))
        prpool = ctx.enter_context(tc.tile_pool(name="pr", bufs=3))
        pp_pool = ctx.enter_context(
            tc.tile_pool(name="ppsum", bufs=1, space="PSUM"))
        # P-matmul also carries the BN sums: prod = [wl*h2 (CF) | h2 | h2^2]
        PWID = CF + 2 * F
        PSPL = []
        off = 0
        while off < PWID:
            w = min(512, PWID - off)
            PSPL.append((off, w))
            off += w
        SSPL = []
        off = 0
        while off < CF:
            w = min(512, CF - off)
            SSPL.append((off, w))
            off += w
        pP = [pp_pool.tile([1, w], F32, tag=f"pP{i}", name=f"pP{i}")
              for i, (_, w) in enumerate(PSPL)]
        pS = [pp_pool.tile([1, w], F32, tag=f"pS{i}", name=f"pS{i}")
              for i, (_, w) in enumerate(SSPL)]
        lw_base = lw_d.ap()

        def tail_chunk(c):
            wl = lwpool.tile([128, CF], F32, tag="wl", name="wl")
            src3 = bass.AP(
                lw_base.tensor, c * 128 * F,
                [[F, 128], [n_nodes * F, n_cls], [1, F]])
            nc.sync.dma_start(
                wl[:].rearrange("p (c f) -> p c f", f=F), src3)
            h2c = h2_sb[:, c * F:(c + 1) * F]
            prod = prpool.tile([128, PWID], F32, tag="prod", name="prod")
            if USE_BIG_PROD:
                h2r = bass.AP(h2c.tensor, h2c.offset,
                              [h2c.ap[0], [0, n_cls], h2c.ap[1]])
                nc.vector.tensor_tensor(
                    prod[:, :CF].rearrange("p (c f) -> p c f", f=F),
                    wl[:].rearrange("p (c f) -> p c f", f=F),
                    h2r, op=ALU.mult)
            else:
                for ci in range(n_cls):
                    nc.vector.tensor_tensor(
                        prod[:, ci * F:(ci + 1) * F],
                        wl[:, ci * F:(ci + 1) * F], h2c, op=ALU.mult)
            nc.vector.tensor_copy(prod[:, CF:CF + F], h2c)
            nc.vector.tensor_tensor(prod[:, CF + F:], h2c, h2c, op=ALU.mult)
            st = (c == 0)
            sp = (c == NS - 1)
            for i, (o, w) in enumerate(PSPL):
                nc.tensor.matmul(pP[i][:], ones_sb[:], prod[:, o:o + w],
                                 start=st, stop=sp, skip_group_check=True)
            for i, (o, w) in enumerate(SSPL):
                nc.tensor.matmul(pS[i][:], ones_sb[:], wl[:, o:o + w],
                                 start=st, stop=sp, skip_group_check=True)

        # ---- two conv layers
        for layer in range(2):
            src_d = xs_d if layer == 0 else h1_d
            w_sb = w1_sb if layer == 0 else w2_sb
            b_sb = b1_sb if layer == 0 else b2_sb
            with ExitStack() as lctx:
                gpool = lctx.enter_context(tc.tile_pool(name=f"g{layer}", bufs=3))
                ohpool = lctx.enter_context(tc.tile_pool(name=f"oh{layer}", bufs=2))
                wpool = lctx.enter_context(tc.tile_pool(name=f"wk{layer}", bufs=3))
                stpool = lctx.enter_context(tc.tile_pool(name=f"st{layer}", bufs=2))
                pa_pool = lctx.enter_context(
                    tc.tile_pool(name=f"pa{layer}", bufs=1, space="PSUM"))
                pb_pool = lctx.enter_context(
                    tc.tile_pool(name=f"pb{layer}", bufs=1, space="PSUM"))
                pt_pool = lctx.enter_context(
                    tc.tile_pool(name=f"pt{layer}", bufs=1, space="PSUM"))

                for g in range(NG):
                    gt = gpool.tile([128, GSL * NBLK * F], F32, tag="gt")
                    nc.gpsimd.dma_gather(
                        out_ap=gt[:].rearrange("p (j f) -> p j f", f=F),
                        in_ap=src_d.ap(),
                        idxs_ap=idx_sb[:, g * IDXW:(g + 1) * IDXW],
                        num_idxs=GSL * EPS,
                        num_idxs_reg=GSL * EPS,
                        elem_size=F,
                        single_packet=False,
                    )
                    if layer == 0:
                        stage = stpool.tile([128, GSL * F], F32, tag="stage")
                    for s_loc in range(GSL):
                        s = g * GSL + s_loc
                        oh = ohpool.tile([128, NBLK * 128], F32, tag="oh")
                        if USE_BIG_OH:
                            a = iota_sb[:]
                            i3 = bass.AP(a.tensor, a.offset,
                                         [a.ap[0], [0, NBLK], a.ap[1]])
                            d = dstloc_sb[:, s * NBLK:(s + 1) * NBLK]
                            d3 = bass.AP(d.tensor, d.offset,
                                         [d.ap[0], d.ap[1], [0, 128]])
                            nc.vector.tensor_tensor(
                                oh[:].rearrange("p (k n) -> p k n", n=128),
                                i3, d3, op=ALU.is_equal)
                        else:
                            for k in range(NBLK):
                                nc.vector.tensor_scalar(
                                    oh[:, k * 128:(k + 1) * 128], iota_sb[:],
                                    dstloc_sb[:, s * NBLK + k:s * NBLK + k + 1],
                                    None, op0=ALU.is_equal)
                        # scatter: aggT[f, n] = sum_e gathered[e, f] * oh[e, n]
                        pa = pa_pool.tile([F, 128], F32, tag="pa")
                        for k in range(NBLK):
                            j = s_loc * NBLK + k
                            nc.tensor.matmul(
                                pa[:], gt[:, j * F:(j + 1) * F],
                                oh[:, k * 128:(k + 1) * 128],
                                start=(k == 0), stop=(k == NBLK - 1))
                        aggT = wpool.tile([F, 128], F32, tag="aggT")
                        nc.vector.tensor_copy(aggT[:], pa[:])
                        # conv: pre[fo, n] = sum_fi W[fi, fo] * aggT[fi, n]
                        pb = pb_pool.tile([F, 128], F32, tag="pb")
                        nc.tensor.matmul(pb[:], w_sb[:], aggT[:])
                        t1 = wpool.tile([F, 128], F32, tag="t1")
                        nc.vector.tensor_copy(t1[:], pb[:])
                        pt = pt_pool.tile([128, F], F32, tag="ptr")
                        nc.tensor.transpose(pt[:], t1[:], ident_sb[:])
                        # epilogue: *rs_in, +bias, (relu * rs_out)
                        t2 = wpool.tile([128, F], F32, tag="t2")
                        nc.vector.tensor_scalar(
                            t2[:], pt[:], rs_in_sb[:, s:s + 1], None,
                            op0=ALU.mult)
                        if layer == 0:
                            t3 = wpool.tile([128, F], F32, tag="t3")
                            nc.vector.tensor_tensor(t3[:], t2[:], b_sb[:],
                                                    op=ALU.add)
                            nc.scalar.activation(
                                stage[:, s_loc * F:(s_loc + 1) * F], t3[:],
                                AF.Relu, scale=rs_out_sb[:, s:s + 1])
                        else:
                            nc.vector.tensor_tensor(
                                h2_sb[:, s * F:(s + 1) * F], t2[:], b_sb[:],
                                op=ALU.add)
                            if INTERLEAVE_TAIL:
                                tail_chunk(s)
                    if layer == 0:
                        dst_ap = h1_d.ap().rearrange("(s p) f -> p s f", p=128)
                        nc.sync.dma_start(
                            dst_ap[:, g * GSL:(g + 1) * GSL, :],
                            stage[:].rearrange("p (s f) -> p s f", f=F))
            if USE_BARRIERS:
                tc.strict_bb_all_engine_barrier()

        # ---- tail: P, S, BN sums (separate phase when not interleaved)
        if True:
            tctx = ctx
            if not INTERLEAVE_TAIL:
                for c in range(NS):
                    tail_chunk(c)

            if debug:
                nc.sync.dma_start(h2_d.ap(), h2_sb[:])
            out_sb = lwpool.tile([1, 2 * CF + 128], F32, tag="outsb")
            # layout: [P(CF) | bn(2F)] from pP splits, then S(CF)
            for i, (o, w) in enumerate(PSPL):
                dst = o
                nc.vector.tensor_copy(out_sb[:, dst:dst + w], pP[i][:])
            for i, (o, w) in enumerate(SSPL):
                nc.vector.tensor_copy(
                    out_sb[:, CF + 2 * F + o:CF + 2 * F + o + w], pS[i][:])
            nc.sync.dma_start(out_d.ap(), out_sb[:])

    nc.compile()
    return nc


_PROGRAM_CACHE = {}


def _get_program(key):
    if key not in _PROGRAM_CACHE:
        _PROGRAM_CACHE[key] = _build_program(*key)
    return _PROGRAM_CACHE[key]


def gcn_forward(x, edge_src, edge_dst, W1, b1, W2, b2, bn_gamma, bn_beta,
                lin_W, lin_b, gsl=None):
    """Full forward pass. x [B, N, F]; returns [B, C]."""
    x = np.asarray(x, np.float32)
    edge_src = np.asarray(edge_src)
    edge_dst = np.asarray(edge_dst)
    W1 = np.asarray(W1, np.float32)
    b1 = np.asarray(b1, np.float32)
    W2 = np.asarray(W2, np.float32)
    b2 = np.asarray(b2, np.float32)
    bn_gamma = np.asarray(bn_gamma, np.float32)
    bn_beta = np.asarray(bn_beta, np.float32)
    lin_W = np.asarray(lin_W, np.float32)
    lin_b = np.asarray(lin_b, np.float32)

    B, N, F = x.shape
    C = lin_W.shape[0]
    NS = N // 128
    n_cores = B

    # padded edges per slice (shared across cores -> same program)
    max_cnt = 1
    for b in range(B):
        cnt = np.bincount(edge_src[b] >> 7, minlength=NS)  # dummy init
        cnt = np.bincount(edge_dst[b] >> 7, minlength=NS)
        max_cnt = max(max_cnt, int(cnt.max()))
    EPS = ((max_cnt + 127) // 128) * 128
    if gsl is None:
        # dma_gather call is validated up to 9216 indices
        gsl = max(1, min(8, NS, 9216 // EPS))
        while NS % gsl:
            gsl -= 1

    nc = _get_program((N, F, EPS, C, n_cores, gsl))

    iota = np.tile(np.arange(128, dtype=np.float32), (128, 1))
    ident = np.eye(F, dtype=np.float32)
    ones = np.ones((128, 1), np.float32)
    b1b = np.tile(b1, (128, 1))
    b2b = np.tile(b2, (128, 1))

    in_maps = []
    for b in range(B):
        idx16, dstloc, rs_out_t, rs_in_t = _prep_graph(
            edge_src[b].astype(np.int64), edge_dst[b].astype(np.int64), N, EPS)
        in_maps.append({
            "x": np.ascontiguousarray(x[b]),
            "idx": idx16,
            "dstloc": dstloc,
            "rs_out": rs_out_t,
            "rs_in": rs_in_t,
            "iota": iota,
            "ident": ident,
            "W1": W1, "W2": W2, "b1b": b1b, "b2b": b2b,
            "ones": ones,
            "lin_W": lin_W,
        })

    res = run_bass_kernel_spmd(nc, in_maps, core_ids=list(range(n_cores)))

    CF = C * F
    P = np.zeros((B, C, F), np.float64)
    s1 = np.zeros(F, np.float64)
    s2 = np.zeros(F, np.float64)
    S = None
    for b in range(B):
        o = res.results[b]["out"][0]
        P[b] = o[:CF].reshape(C, F)
        s1 += o[CF:CF + F]
        s2 += o[CF + F:CF + 2 * F]
        if S is None:
            S = o[CF + 2 * F:2 * CF + 2 * F].reshape(C, F).astype(np.float64)

    cnt = B * N
    mean = s1 / cnt
    var = s2 / cnt - mean * mean
    a = bn_gamma / np.sqrt(var + BN_EPS)
    d = bn_beta - mean * a
    out = (P * a[None, None, :]).sum(-1) + (S * d[None, :]).sum(-1)[None, :] \
        + lin_b[None, :]
    return out.astype(np.float32)


def kernel(**inputs):
    return gcn_forward(
        inputs["x"], inputs["edge_src"], inputs["edge_dst"],
        inputs["W1"], inputs["b1"], inputs["W2"], inputs["b2"],
        inputs["bn_gamma"], inputs["bn_beta"], inputs["lin_W"], inputs["lin_b"])


# revision 14
# speedup vs baseline: 1.0781x; 1.0781x over previous
"""GCN (2x GraphConv + BatchNorm + Linear) forward on 8 Trainium2 NeuronCores.

Sharding: data-parallel over the batch axis -- each core owns one whole graph,
so the gather/segment-sum stays core-local.  The big lin_W contraction is
reformulated per-channel:

  out[b,c] = sum_f a[f] * P[b,c,f] + sum_f d[f] * S[c,f] + lin_b[c]

where P[b,c,f] = sum_n h2[b,n,f] * lin_W[c, n*F+f], S[c,f] = sum_n lin_W[c,n*F+f],
and (a, d) are the BatchNorm affine coefficients derived from global mean/var.
Each core computes its graph's h2, BN partial sums (sum h2, sum h2^2), P and S
on device; the host combines the tiny per-core results.

Device algorithm per core (all fp32):
  prep : load x, scale rows by rsqrt(deg_out), store xs to HBM
  layer: for each 128-node dst slice: dma_gather xs[src] rows (edges sorted by
         dst slice, padded per-slice), build one-hot matrices from local dst
         indices on DVE, scatter via PE matmul (gathered^T @ onehot -> agg^T),
         conv matmul with W, PE transpose, scale by rsqrt(deg_in), +bias,
         (relu * rsqrt(deg_out) for layer 1) -> h1 to HBM / h2 stays in SBUF
  tail : stream lin_W, accumulate P, S and BN sums via ones-matmuls in PSUM.
"""

import math
import os
from contextlib import ExitStack

import numpy as np

import concourse.bass as bass
import concourse.tile as tile
from concourse import bacc, mybir
from concourse.bass_utils import run_bass_kernel_spmd

F32 = mybir.dt.float32
I16 = mybir.dt.int16
AF = mybir.ActivationFunctionType
ALU = mybir.AluOpType

BN_EPS = 1e-5

# Set to False to fall back to per-block one-hot builds (no stride-0 APs).
USE_BIG_OH = True
USE_BIG_PROD = True
USE_BARRIERS = True
INTERLEAVE_TAIL = False


# ---------------------------------------------------------------- host prep

def _prep_graph(src, dst, n_nodes, eps):
    """Sort edges by (dst slice, src), pad each slice to `eps` edges.

    Returns (idx16, dstloc, rs_out, rs_in):
      idx16  [128, npad//16] int16  gather indices, edge i at [i%16, i//16]
      dstloc [128, npad//128] f32   local dst (0..127) per edge, 128.0 = pad
      rs_out [128, nslice] f32      rsqrt(max(out_deg,1)),  n = s*128 + p
      rs_in  [128, nslice] f32      rsqrt(max(in_deg,1))
    """
    nslice = n_nodes // 128
    deg_out = np.bincount(src, minlength=n_nodes).astype(np.float32)
    deg_in = np.bincount(dst, minlength=n_nodes).astype(np.float32)
    rs_out = (1.0 / np.sqrt(np.maximum(deg_out, 1.0))).astype(np.float32)
    rs_in = (1.0 / np.sqrt(np.maximum(deg_in, 1.0))).astype(np.float32)
    rs_out_t = rs_out.reshape(nslice, 128).T.copy()
    rs_in_t = rs_in.reshape(nslice, 128).T.copy()

    sl = dst >> 7
    order = np.lexsort((src, sl))
    src_s = src[order].astype(np.int64)
    dst_s = dst[order].astype(np.int64)
    sl_s = sl[order]
    counts = np.bincount(sl_s, minlength=nslice)
    assert counts.max() <= eps, (counts.max(), eps)

    npad = nslice * eps
    src_pad = np.zeros(npad, np.int16)
    dstloc_pad = np.full(npad, 128.0, np.float32)
    starts = np.zeros(nslice + 1, np.int64)
    np.cumsum(counts, out=starts[1:])
    # position of edge k (sorted) inside padded layout
    within = np.arange(len(src_s)) - starts[sl_s]
    pos = sl_s * eps + within
    src_pad[pos] = src_s.astype(np.int16)
    dstloc_pad[pos] = (dst_s & 127).astype(np.float32)

    idx16 = np.tile(src_pad.reshape(-1, 16).T, (8, 1))  # replicated across Q7 cores
    dstloc = dstloc_pad.reshape(-1, 128).T.copy()  # [128, npad//128]
    return idx16, dstloc, rs_out_t, rs_in_t


# ---------------------------------------------------------------- device build

def _build_program(n_nodes, feat, n_edges_pad_per_slice, n_cls, n_cores, gsl):
    """Build the Bass program. Returns (nc, out_name)."""
    NS = n_nodes // 128          # dst slices == node chunks
    F = feat
    EPS = n_edges_pad_per_slice  # padded edges per slice, multiple of 128
    NBLK = EPS // 128            # 128-edge blocks per slice
    NPAD = NS * EPS
    CF = n_cls * F
    GSL = gsl                    # slices per dma_gather call
    assert NS % GSL == 0
    NG = NS // GSL
    IDXW = GSL * EPS // 16       # idx columns per gather call

    nc = bacc.Bacc(
        "TRN2", target_bir_lowering=False, debug=False, num_devices=n_cores
    )

    x_d = nc.dram_tensor("x", [n_nodes, F], F32, kind="ExternalInput")
    idx_d = nc.dram_tensor("idx", [128, NPAD // 16], I16, kind="ExternalInput")
    dstloc_d = nc.dram_tensor("dstloc", [128, NPAD // 128], F32, kind="ExternalInput")
    rs_out_d = nc.dram_tensor("rs_out", [128, NS], F32, kind="ExternalInput")
    rs_in_d = nc.dram_tensor("rs_in", [128, NS], F32, kind="ExternalInput")
    iota_d = nc.dram_tensor("iota", [128, 128], F32, kind="ExternalInput")
    ident_d = nc.dram_tensor("ident", [F, F], F32, kind="ExternalInput")
    w1_d = nc.dram_tensor("W1", [F, F], F32, kind="ExternalInput")
    w2_d = nc.dram_tensor("W2", [F, F], F32, kind="ExternalInput")
    b1_d = nc.dram_tensor("b1b", [128, F], F32, kind="ExternalInput")
    b2_d = nc.dram_tensor("b2b", [128, F], F32, kind="ExternalInput")
    ones_d = nc.dram_tensor("ones", [128, 1], F32, kind="ExternalInput")
    lw_d = nc.dram_tensor("lin_W", [n_cls, n_nodes * F], F32, kind="ExternalInput")

    out_d = nc.dram_tensor("out", [1, 2 * CF + 128], F32, kind="ExternalOutput")

    debug = bool(os.environ.get("GCN_DEBUG"))
    kind_i = "ExternalOutput" if debug else "Internal"
    xs_d = nc.dram_tensor("xs_i", [n_nodes, F], F32, kind=kind_i)
    h1_d = nc.dram_tensor("h1_i", [n_nodes, F], F32, kind=kind_i)
    h2_d = (nc.dram_tensor("h2_i", [128, NS * F], F32, kind="ExternalOutput")
            if debug else None)

    with tile.TileContext(nc) as tc, ExitStack() as ctx:
        cpool = ctx.enter_context(tc.tile_pool(name="const", bufs=1))
        iota_sb = cpool.tile([128, 128], F32, tag="iota")
        ident_sb = cpool.tile([F, F], F32, tag="ident")
        w1_sb = cpool.tile([F, F], F32, tag="w1")
        w2_sb = cpool.tile([F, F], F32, tag="w2")
        b1_sb = cpool.tile([128, F], F32, tag="b1")
        b2_sb = cpool.tile([128, F], F32, tag="b2")
        ones_sb = cpool.tile([128, 1], F32, tag="ones")
        rs_out_sb = cpool.tile([128, NS], F32, tag="rso")
        rs_in_sb = cpool.tile([128, NS], F32, tag="rsi")
        dstloc_sb = cpool.tile([128, NPAD // 128], F32, tag="dstloc")
        idx_sb = cpool.tile([128, NPAD // 16], I16, tag="idx")
        h2_sb = cpool.tile([128, NS * F], F32, tag="h2")

        for t, d in [
            (iota_sb, iota_d), (ident_sb, ident_d), (w1_sb, w1_d),
            (w2_sb, w2_d), (b1_sb, b1_d), (b2_sb, b2_d), (ones_sb, ones_d),
            (rs_out_sb, rs_out_d), (rs_in_sb, rs_in_d),
            (dstloc_sb, dstloc_d), (idx_sb, idx_d),
        ]:
            nc.sync.dma_start(t[:], d.ap())

        # ---- prep: xs = x * rs_out (per node), store to HBM
        with tc.tile_pool(name="prep", bufs=1) as ppool:
            x_sb = ppool.tile([128, NS * F], F32, tag="xsb")
            nc.sync.dma_start(
                x_sb[:].rearrange("p (s f) -> p s f", f=F),
                x_d.ap().rearrange("(s p) f -> p s f", p=128),
            )
            for s in range(NS):
                nc.vector.tensor_scalar(
                    x_sb[:, s * F:(s + 1) * F], x_sb[:, s * F:(s + 1) * F],
                    rs_out_sb[:, s:s + 1], None, op0=ALU.mult,
                )
            nc.sync.dma_start(
                xs_d.ap().rearrange("(s p) f -> p s f", p=128),
                x_sb[:].rearrange("p (s f) -> p s f", f=F),
            )

        if USE_BARRIERS:
            tc.strict_bb_all_engine_barrier()

        # ---- tail pools (P, S, BN accumulation), usable inside layer 2
        lwpool = ctx.enter_context(tc.tile_pool(name="lw", bufs=3))
        prpool = ctx.enter_context(tc.tile_pool(name="pr", bufs=3))
        pp_pool = ctx.enter_context(
            tc.tile_pool(name="ppsum", bufs=1, space="PSUM"))
        # P-matmul also carries the BN sums: prod = [wl*h2 (CF) | h2 | h2^2]
        PWID = CF + 2 * F
        PSPL = []
        off = 0
        while off < PWID:
            w = min(512, PWID - off)
            PSPL.append((off, w))
            off += w
        SSPL = []
        off = 0
        while off < CF:
            w = min(512, CF - off)
            SSPL.append((off, w))
            off += w
        pP = [pp_pool.tile([1, w], F32, tag=f"pP{i}", name=f"pP{i}")
              for i, (_, w) in enumerate(PSPL)]
        pS = [pp_pool.tile([1, w], F32, tag=f"pS{i}", name=f"pS{i}")
              for i, (_, w) in enumerate(SSPL)]
        lw_base = lw_d.ap()

        def tail_chunk(c):
            wl = lwpool.tile([128, CF], F32, tag="wl", name="wl")
            src3 = bass.AP(
                lw_base.tensor, c * 128 * F,
                [[F, 128], [n_nodes * F, n_cls], [1, F]])
            nc.sync.dma_start(
                wl[:].rearrange("p (c f) -> p c f", f=F), src3)
            h2c = h2_sb[:, c * F:(c + 1) * F]
            prod = prpool.tile([128, PWID], F32, tag="prod", name="prod")
            if USE_BIG_PROD:
                h2r = bass.AP(h2c.tensor, h2c.offset,
                              [h2c.ap[0], [0, n_cls], h2c.ap[1]])
                nc.vector.tensor_tensor(
                    prod[:, :CF].rearrange("p (c f) -> p c f", f=F),
                    wl[:].rearrange("p (c f) -> p c f", f=F),
                    h2r, op=ALU.mult)
            else:
                for ci in range(n_cls):
                    nc.vector.tensor_tensor(
                        prod[:, ci * F:(ci + 1) * F],
                        wl[:, ci * F:(ci + 1) * F], h2c, op=ALU.mult)
            nc.vector.tensor_copy(prod[:, CF:CF + F], h2c)
            nc.vector.tensor_tensor(prod[:, CF + F:], h2c, h2c, op=ALU.mult)
            st = (c == 0)
            sp = (c == NS - 1)
            for i, (o, w) in enumerate(PSPL):
                nc.tensor.matmul(pP[i][:], ones_sb[:], prod[:, o:o + w],
                                 start=st, stop=sp, skip_group_check=True)
            for i, (o, w) in enumerate(SSPL):
                nc.tensor.matmul(pS[i][:], ones_sb[:], wl[:, o:o + w],
                                 start=st, stop=sp, skip_group_check=True)

        # ---- two conv layers
        for layer in range(2):
            src_d = xs_d if layer == 0 else h1_d
            w_sb = w1_sb if layer == 0 else w2_sb
            b_sb = b1_sb if layer == 0 else b2_sb
            with ExitStack() as lctx:
                gpool = lctx.enter_context(tc.tile_pool(name=f"g{layer}", bufs=3))
                ohpool = lctx.enter_context(tc.tile_pool(name=f"oh{layer}", bufs=2))
                wpool = lctx.enter_context(tc.tile_pool(name=f"wk{layer}", bufs=3))
                stpool = lctx.enter_context(tc.tile_pool(name=f"st{layer}", bufs=2))
                pa_pool = lctx.enter_context(
                    tc.tile_pool(name=f"pa{layer}", bufs=2, space="PSUM"))
                pbt_pool = lctx.enter_context(
                    tc.tile_pool(name=f"pbt{layer}", bufs=2, space="PSUM"))
                pb_pool = pt_pool = pbt_pool

                for g in range(NG):
                    gt = gpool.tile([128, GSL * NBLK * F], F32, tag="gt")
                    nc.gpsimd.dma_gather(
                        out_ap=gt[:].rearrange("p (j f) -> p j f", f=F),
                        in_ap=src_d.ap(),
                        idxs_ap=idx_sb[:, g * IDXW:(g + 1) * IDXW],
                        num_idxs=GSL * EPS,
                        num_idxs_reg=GSL * EPS,
                        elem_size=F,
                        single_packet=False,
                    )
                    if layer == 0:
                        stage = stpool.tile([128, GSL * F], F32, tag="stage")
                    for s_loc in range(GSL):
                        s = g * GSL + s_loc
                        oh = ohpool.tile([128, NBLK * 128], F32, tag="oh")
                        if USE_BIG_OH:
                            a = iota_sb[:]
                            i3 = bass.AP(a.tensor, a.offset,
                                         [a.ap[0], [0, NBLK], a.ap[1]])
                            d = dstloc_sb[:, s * NBLK:(s + 1) * NBLK]
                            d3 = bass.AP(d.tensor, d.offset,
                                         [d.ap[0], d.ap[1], [0, 128]])
                            nc.vector.tensor_tensor(
                                oh[:].rearrange("p (k n) -> p k n", n=128),
                                i3, d3, op=ALU.is_equal)
                        else:
                            for k in range(NBLK):
                                nc.vector.tensor_scalar(
                                    oh[:, k * 128:(k + 1) * 128], iota_sb[:],
                                    dstloc_sb[:, s * NBLK + k:s * NBLK + k + 1],
                                    None, op0=ALU.is_equal)
                        # scatter: aggT[f, n] = sum_e gathered[e, f] * oh[e, n]
                        pa = pa_pool.tile([F, 128], F32, tag="pa")
                        for k in range(NBLK):
                            j = s_loc * NBLK + k
                            nc.tensor.matmul(
                                pa[:], gt[:, j * F:(j + 1) * F],
                                oh[:, k * 128:(k + 1) * 128],
                                start=(k == 0), stop=(k == NBLK - 1))
                        aggT = wpool.tile([F, 128], F32, tag="aggT")
                        nc.vector.tensor_copy(aggT[:], pa[:])
                        # conv: pre[fo, n] = sum_fi W[fi, fo] * aggT[fi, n]
                        pb = pb_pool.tile([F, 128], F32, tag="pbt", name="pb")
                        nc.tensor.matmul(pb[:], w_sb[:], aggT[:])
                        t1 = wpool.tile([F, 128], F32, tag="t1")
                        nc.vector.tensor_copy(t1[:], pb[:])
                        pt = pt_pool.tile([128, F], F32, tag="pbt", name="pt")
                        nc.tensor.transpose(pt[:], t1[:], ident_sb[:])
                        # epilogue: *rs_in, +bias, (relu * rs_out)
                        t2 = wpool.tile([128, F], F32, tag="t2")
                        nc.vector.tensor_scalar(
                            t2[:], pt[:], rs_in_sb[:, s:s + 1], None,
                            op0=ALU.mult)
                        if layer == 0:
                            t3 = wpool.tile([128, F], F32, tag="t3")
                            nc.vector.tensor_tensor(t3[:], t2[:], b_sb[:],
                                                    op=ALU.add)
                            nc.scalar.activation(
                                stage[:, s_loc * F:(s_loc + 1) * F], t3[:],
                                AF.Relu, scale=rs_out_sb[:, s:s + 1])
                        else:
                            nc.vector.tensor_tensor(
                                h2_sb[:, s * F:(s + 1) * F], t2[:], b_sb[:],
                                op=ALU.add)
                            if INTERLEAVE_TAIL:
                                tail_chunk(s)
                    if layer == 0:
                        dst_ap = h1_d.ap().rearrange("(s p) f -> p s f", p=128)
                        nc.sync.dma_start(
                            dst_ap[:, g * GSL:(g + 1) * GSL, :],
                            stage[:].rearrange("p (s f) -> p s f", f=F))
            if USE_BARRIERS:
                tc.strict_bb_all_engine_barrier()

        # ---- tail: P, S, BN sums (separate phase when not interleaved)
        if True:
            tctx = ctx
            if not INTERLEAVE_TAIL:
                for c in range(NS):
                    tail_chunk(c)

            if debug:
                nc.sync.dma_start(h2_d.ap(), h2_sb[:])
            out_sb = lwpool.tile([1, 2 * CF + 128], F32, tag="outsb")
            # layout: [P(CF) | bn(2F)] from pP splits, then S(CF)
            for i, (o, w) in enumerate(PSPL):
                dst = o
                nc.vector.tensor_copy(out_sb[:, dst:dst + w], pP[i][:])
            for i, (o, w) in enumerate(SSPL):
                nc.vector.tensor_copy(
                    out_sb[:, CF + 2 * F + o:CF + 2 * F + o + w], pS[i][:])
            nc.sync.dma_start(out_d.ap(), out_sb[:])

    nc.compile()
    return nc


_PROGRAM_CACHE = {}


def _get_program(key):
    if key not in _PROGRAM_CACHE:
        _PROGRAM_CACHE[key] = _build_program(*key)
    return _PROGRAM_CACHE[key]


def gcn_forward(x, edge_src, edge_dst, W1, b1, W2, b2, bn_gamma, bn_beta,
                lin_W, lin_b, gsl=None):
    """Full forward pass. x [B, N, F]; returns [B, C]."""
    x = np.asarray(x, np.float32)
    edge_src = np.asarray(edge_src)
    edge_dst = np.asarray(edge_dst)
    W1 = np.asarray(W1, np.float32)
    b1 = np.asarray(b1, np.float32)
    W2 = np.asarray(W2, np.float32)
    b2 = np.asarray(b2, np.float32)
    bn_gamma = np.asarray(bn_gamma, np.float32)
    bn_beta = np.asarray(bn_beta, np.float32)
    lin_W = np.asarray(lin_W, np.float32)
    lin_b = np.asarray(lin_b, np.float32)

    B, N, F = x.shape
    C = lin_W.shape[0]
    NS = N // 128
    n_cores = B

    # padded edges per slice (shared across cores -> same program)
    max_cnt = 1
    for b in range(B):
        cnt = np.bincount(edge_src[b] >> 7, minlength=NS)  # dummy init
        cnt = np.bincount(edge_dst[b] >> 7, minlength=NS)
        max_cnt = max(max_cnt, int(cnt.max()))
    EPS = ((max_cnt + 127) // 128) * 128
    if gsl is None:
        # dma_gather call is validated up to 9216 indices
        gsl = max(1, min(8, NS, 9216 // EPS))
        while NS % gsl:
            gsl -= 1

    nc = _get_program((N, F, EPS, C, n_cores, gsl))

    iota = np.tile(np.arange(128, dtype=np.float32), (128, 1))
    ident = np.eye(F, dtype=np.float32)
    ones = np.ones((128, 1), np.float32)
    b1b = np.tile(b1, (128, 1))
    b2b = np.tile(b2, (128, 1))

    in_maps = []
    for b in range(B):
        idx16, dstloc, rs_out_t, rs_in_t = _prep_graph(
            edge_src[b].astype(np.int64), edge_dst[b].astype(np.int64), N, EPS)
        in_maps.append({
            "x": np.ascontiguousarray(x[b]),
            "idx": idx16,
            "dstloc": dstloc,
            "rs_out": rs_out_t,
            "rs_in": rs_in_t,
            "iota": iota,
            "ident": ident,
            "W1": W1, "W2": W2, "b1b": b1b, "b2b": b2b,
            "ones": ones,
            "lin_W": lin_W,
        })

    res = run_bass_kernel_spmd(nc, in_maps, core_ids=list(range(n_cores)))

    CF = C * F
    P = np.zeros((B, C, F), np.float64)
    s1 = np.zeros(F, np.float64)
    s2 = np.zeros(F, np.float64)
    S = None
    for b in range(B):
        o = res.results[b]["out"][0]
        P[b] = o[:CF].reshape(C, F)
        s1 += o[CF:CF + F]
        s2 += o[CF + F:CF + 2 * F]
        if S is None:
            S = o[CF + 2 * F:2 * CF + 2 * F].reshape(C, F).astype(np.float64)

    cnt = B * N
    mean = s1 / cnt
    var = s2 / cnt - mean * mean
    a = bn_gamma / np.sqrt(var + BN_EPS)
    d = bn_beta - mean * a
    out = (P * a[None, None, :]).sum(-1) + (S * d[None, :]).sum(-1)[None, :] \
        + lin_b[None, :]
    return out.astype(np.float32)


def kernel(**inputs):
    return gcn_forward(
        inputs["x"], inputs["edge_src"], inputs["edge_dst"],
        inputs["W1"], inputs["b1"], inputs["W2"], inputs["b2"],
        inputs["bn_gamma"], inputs["bn_beta"], inputs["lin_W"], inputs["lin_b"])


# revision 15
# speedup vs baseline: 1.0883x; 1.0095x over previous
"""GCN (2x GraphConv + BatchNorm + Linear) forward on 8 Trainium2 NeuronCores.

Sharding: data-parallel over the batch axis -- each core owns one whole graph,
so the gather/segment-sum stays core-local.  The big lin_W contraction is
reformulated per-channel:

  out[b,c] = sum_f a[f] * P[b,c,f] + sum_f d[f] * S[c,f] + lin_b[c]

where P[b,c,f] = sum_n h2[b,n,f] * lin_W[c, n*F+f], S[c,f] = sum_n lin_W[c,n*F+f],
and (a, d) are the BatchNorm affine coefficients derived from global mean/var.
Each core computes its graph's h2, BN partial sums (sum h2, sum h2^2), P and S
on device; the host combines the tiny per-core results.

Device algorithm per core (all fp32):
  prep : load x, scale rows by rsqrt(deg_out), store xs to HBM
  layer: for each 128-node dst slice: dma_gather xs[src] rows (edges sorted by
         dst slice, padded per-slice), build one-hot matrices from local dst
         indices on DVE, scatter via PE matmul (gathered^T @ onehot -> agg^T),
         conv matmul with W, PE transpose, scale by rsqrt(deg_in), +bias,
         (relu * rsqrt(deg_out) for layer 1) -> h1 to HBM / h2 stays in SBUF
  tail : stream lin_W, accumulate P, S and BN sums via ones-matmuls in PSUM.
"""

import math
import os
from contextlib import ExitStack

import numpy as np

import concourse.bass as bass
import concourse.tile as tile
from concourse import bacc, mybir
from concourse.bass_utils import run_bass_kernel_spmd

F32 = mybir.dt.float32
I16 = mybir.dt.int16
AF = mybir.ActivationFunctionType
ALU = mybir.AluOpType

BN_EPS = 1e-5

# Set to False to fall back to per-block one-hot builds (no stride-0 APs).
USE_BIG_OH = True
USE_BIG_PROD = True
USE_BARRIERS = True
INTERLEAVE_TAIL = False


# ---------------------------------------------------------------- host prep

def _prep_graph(src, dst, n_nodes, eps):
    """Sort edges by (dst slice, src), pad each slice to `eps` edges.

    Returns (idx16, dstloc, rs_out, rs_in):
      idx16  [128, npad//16] int16  gather indices, edge i at [i%16, i//16]
      dstloc [128, npad//128] f32   local dst (0..127) per edge, 128.0 = pad
      rs_out [128, nslice] f32      rsqrt(max(out_deg,1)),  n = s*128 + p
      rs_in  [128, nslice] f32      rsqrt(max(in_deg,1))
    """
    nslice = n_nodes // 128
    deg_out = np.bincount(src, minlength=n_nodes).astype(np.float32)
    deg_in = np.bincount(dst, minlength=n_nodes).astype(np.float32)
    rs_out = (1.0 / np.sqrt(np.maximum(deg_out, 1.0))).astype(np.float32)
    rs_in = (1.0 / np.sqrt(np.maximum(deg_in, 1.0))).astype(np.float32)
    rs_out_t = rs_out.reshape(nslice, 128).T.copy()
    rs_in_t = rs_in.reshape(nslice, 128).T.copy()

    sl = dst >> 7
    order = np.lexsort((src, sl))
    src_s = src[order].astype(np.int64)
    dst_s = dst[order].astype(np.int64)
    sl_s = sl[order]
    counts = np.bincount(sl_s, minlength=nslice)
    assert counts.max() <= eps, (counts.max(), eps)

    npad = nslice * eps
    src_pad = np.zeros(npad, np.int16)
    dstloc_pad = np.full(npad, 128.0, np.float32)
    starts = np.zeros(nslice + 1, np.int64)
    np.cumsum(counts, out=starts[1:])
    # position of edge k (sorted) inside padded layout
    within = np.arange(len(src_s)) - starts[sl_s]
    pos = sl_s * eps + within
    src_pad[pos] = src_s.astype(np.int16)
    dstloc_pad[pos] = (dst_s & 127).astype(np.float32)

    idx16 = np.tile(src_pad.reshape(-1, 16).T, (8, 1))  # replicated across Q7 cores
    dstloc = dstloc_pad.reshape(-1, 128).T.copy()  # [128, npad//128]
    return idx16, dstloc, rs_out_t, rs_in_t


# ---------------------------------------------------------------- device build

def _build_program(n_nodes, feat, n_edges_pad_per_slice, n_cls, n_cores, gsl):
    """Build the Bass program. Returns (nc, out_name)."""
    NS = n_nodes // 128          # dst slices == node chunks
    F = feat
    EPS = n_edges_pad_per_slice  # padded edges per slice, multiple of 128
    NBLK = EPS // 128            # 128-edge blocks per slice
    NPAD = NS * EPS
    CF = n_cls * F
    GSL = gsl                    # slices per dma_gather call
    assert NS % GSL == 0
    NG = NS // GSL
    IDXW = GSL * EPS // 16       # idx columns per gather call

    nc = bacc.Bacc(
        "TRN2", target_bir_lowering=False, debug=False, num_devices=n_cores
    )

    x_d = nc.dram_tensor("x", [n_nodes, F], F32, kind="ExternalInput")
    idx_d = nc.dram_tensor("idx", [128, NPAD // 16], I16, kind="ExternalInput")
    dstloc_d = nc.dram_tensor("dstloc", [128, NPAD // 128], F32, kind="ExternalInput")
    rs_out_d = nc.dram_tensor("rs_out", [128, NS], F32, kind="ExternalInput")
    rs_in_d = nc.dram_tensor("rs_in", [128, NS], F32, kind="ExternalInput")
    iota_d = nc.dram_tensor("iota", [128, 128], F32, kind="ExternalInput")
    ident_d = nc.dram_tensor("ident", [F, F], F32, kind="ExternalInput")
    w1_d = nc.dram_tensor("W1", [F, F], F32, kind="ExternalInput")
    w2_d = nc.dram_tensor("W2", [F, F], F32, kind="ExternalInput")
    b1_d = nc.dram_tensor("b1b", [128, F], F32, kind="ExternalInput")
    b2_d = nc.dram_tensor("b2b", [128, F], F32, kind="ExternalInput")
    ones_d = nc.dram_tensor("ones", [128, 1], F32, kind="ExternalInput")
    lw_d = nc.dram_tensor("lin_W", [n_cls, n_nodes * F], F32, kind="ExternalInput")

    out_d = nc.dram_tensor("out", [1, 2 * CF + 128], F32, kind="ExternalOutput")

    debug = bool(os.environ.get("GCN_DEBUG"))
    kind_i = "ExternalOutput" if debug else "Internal"
    xs_d = nc.dram_tensor("xs_i", [n_nodes, F], F32, kind=kind_i)
    h1_d = nc.dram_tensor("h1_i", [n_nodes, F], F32, kind=kind_i)
    h2_d = (nc.dram_tensor("h2_i", [128, NS * F], F32, kind="ExternalOutput")
            if debug else None)

    with tile.TileContext(nc) as tc, ExitStack() as ctx:
        cpool = ctx.enter_context(tc.tile_pool(name="const", bufs=1))
        iota_sb = cpool.tile([128, 128], F32, tag="iota")
        ident_sb = cpool.tile([F, F], F32, tag="ident")
        w1_sb = cpool.tile([F, F], F32, tag="w1")
        w2_sb = cpool.tile([F, F], F32, tag="w2")
        b1_sb = cpool.tile([128, F], F32, tag="b1")
        b2_sb = cpool.tile([128, F], F32, tag="b2")
        ones_sb = cpool.tile([128, 1], F32, tag="ones")
        rs_out_sb = cpool.tile([128, NS], F32, tag="rso")
        rs_in_sb = cpool.tile([128, NS], F32, tag="rsi")
        dstloc_sb = cpool.tile([128, NPAD // 128], F32, tag="dstloc")
        idx_sb = cpool.tile([128, NPAD // 16], I16, tag="idx")
        h2_sb = cpool.tile([128, NS * F], F32, tag="h2")

        for t, d in [
            (iota_sb, iota_d), (ident_sb, ident_d), (w1_sb, w1_d),
            (w2_sb, w2_d), (b1_sb, b1_d), (b2_sb, b2_d), (ones_sb, ones_d),
            (rs_out_sb, rs_out_d), (rs_in_sb, rs_in_d),
            (dstloc_sb, dstloc_d), (idx_sb, idx_d),
        ]:
            nc.sync.dma_start(t[:], d.ap())

        # ---- prep: xs = x * rs_out (per node), store to HBM
        with tc.tile_pool(name="prep", bufs=1) as ppool:
            x_sb = ppool.tile([128, NS * F], F32, tag="xsb")
            nc.sync.dma_start(
                x_sb[:].rearrange("p (s f) -> p s f", f=F),
                x_d.ap().rearrange("(s p) f -> p s f", p=128),
            )
            for s in range(NS):
                nc.vector.tensor_scalar(
                    x_sb[:, s * F:(s + 1) * F], x_sb[:, s * F:(s + 1) * F],
                    rs_out_sb[:, s:s + 1], None, op0=ALU.mult,
                )
            nc.sync.dma_start(
                xs_d.ap().rearrange("(s p) f -> p s f", p=128),
                x_sb[:].rearrange("p (s f) -> p s f", f=F),
            )

        if USE_BARRIERS:
            tc.strict_bb_all_engine_barrier()

        # ---- tail pools (P, S, BN accumulation), usable inside layer 2
        lwpool = ctx.enter_context(tc.tile_pool(name="lw", bufs=3))
        prpool = ctx.enter_context(tc.tile_pool(name="pr", bufs=3))
        pp_pool = ctx.enter_context(
            tc.tile_pool(name="ppsum", bufs=1, space="PSUM"))
        # P-matmul also carries the BN sums: prod = [wl*h2 (CF) | h2 | h2^2]
        PWID = CF + 2 * F
        PSPL = []
        off = 0
        while off < PWID:
            w = min(512, PWID - off)
            PSPL.append((off, w))
            off += w
        SSPL = []
        off = 0
        while off < CF:
            w = min(512, CF - off)
            SSPL.append((off, w))
            off += w
        pP = [pp_pool.tile([1, w], F32, tag=f"pP{i}", name=f"pP{i}")
              for i, (_, w) in enumerate(PSPL)]
        pS = [pp_pool.tile([1, w], F32, tag=f"pS{i}", name=f"pS{i}")
              for i, (_, w) in enumerate(SSPL)]
        lw_base = lw_d.ap()

        def tail_chunk(c):
            wl = lwpool.tile([128, CF], F32, tag="wl", name="wl")
            src3 = bass.AP(
                lw_base.tensor, c * 128 * F,
                [[F, 128], [n_nodes * F, n_cls], [1, F]])
            nc.sync.dma_start(
                wl[:].rearrange("p (c f) -> p c f", f=F), src3)
            h2c = h2_sb[:, c * F:(c + 1) * F]
            prod = prpool.tile([128, PWID], F32, tag="prod", name="prod")
            if USE_BIG_PROD:
                h2r = bass.AP(h2c.tensor, h2c.offset,
                              [h2c.ap[0], [0, n_cls], h2c.ap[1]])
                nc.vector.tensor_tensor(
                    prod[:, :CF].rearrange("p (c f) -> p c f", f=F),
                    wl[:].rearrange("p (c f) -> p c f", f=F),
                    h2r, op=ALU.mult)
            else:
                for ci in range(n_cls):
                    nc.vector.tensor_tensor(
                        prod[:, ci * F:(ci + 1) * F],
                        wl[:, ci * F:(ci + 1) * F], h2c, op=ALU.mult)
            nc.vector.tensor_copy(prod[:, CF:CF + F], h2c)
            nc.vector.tensor_tensor(prod[:, CF + F:], h2c, h2c, op=ALU.mult)
            st = (c == 0)
            sp = (c == NS - 1)
            for i, (o, w) in enumerate(PSPL):
                nc.tensor.matmul(pP[i][:], ones_sb[:], prod[:, o:o + w],
                                 start=st, stop=sp, skip_group_check=True)
            for i, (o, w) in enumerate(SSPL):
                nc.tensor.matmul(pS[i][:], ones_sb[:], wl[:, o:o + w],
                                 start=st, stop=sp, skip_group_check=True)

        # ---- two conv layers
        for layer in range(2):
            src_d = xs_d if layer == 0 else h1_d
            w_sb = w1_sb if layer == 0 else w2_sb
            b_sb = b1_sb if layer == 0 else b2_sb
            with ExitStack() as lctx:
                gpool = lctx.enter_context(tc.tile_pool(name=f"g{layer}", bufs=3))
                ohpool = lctx.enter_context(tc.tile_pool(name=f"oh{layer}", bufs=2))
                wpool = lctx.enter_context(tc.tile_pool(name=f"wk{layer}", bufs=3))
                stpool = lctx.enter_context(tc.tile_pool(name=f"st{layer}", bufs=2))
                pa_pool = lctx.enter_context(
                    tc.tile_pool(name=f"pa{layer}", bufs=2, space="PSUM"))
                pbt_pool = lctx.enter_context(
                    tc.tile_pool(name=f"pbt{layer}", bufs=2, space="PSUM"))
                pb_pool = pt_pool = pbt_pool

                for g in range(NG):
                    gt = gpool.tile([128, GSL * NBLK * F], F32, tag="gt")
                    nc.gpsimd.dma_gather(
                        out_ap=gt[:].rearrange("p (j f) -> p j f", f=F),
                        in_ap=src_d.ap(),
                        idxs_ap=idx_sb[:, g * IDXW:(g + 1) * IDXW],
                        num_idxs=GSL * EPS,
                        num_idxs_reg=GSL * EPS,
                        elem_size=F,
                        single_packet=False,
                    )
                    if layer == 0:
                        stage = stpool.tile([128, GSL * F], F32, tag="stage")
                    for s_loc in range(GSL):
                        s = g * GSL + s_loc
                        oh = ohpool.tile([128, NBLK * 128], F32, tag="oh")
                        if USE_BIG_OH:
                            a = iota_sb[:]
                            i3 = bass.AP(a.tensor, a.offset,
                                         [a.ap[0], [0, NBLK], a.ap[1]])
                            d = dstloc_sb[:, s * NBLK:(s + 1) * NBLK]
                            d3 = bass.AP(d.tensor, d.offset,
                                         [d.ap[0], d.ap[1], [0, 128]])
                            nc.vector.tensor_tensor(
                                oh[:].rearrange("p (k n) -> p k n", n=128),
                                i3, d3, op=ALU.is_equal)
                        else:
                            for k in range(NBLK):
                                nc.vector.tensor_scalar(
                                    oh[:, k * 128:(k + 1) * 128], iota_sb[:],
                                    dstloc_sb[:, s * NBLK + k:s * NBLK + k + 1],
                                    None, op0=ALU.is_equal)
                        # scatter: aggT[f, n] = sum_e gathered[e, f] * oh[e, n]
                        pa = pa_pool.tile([F, 128], F32, tag="pa")
                        for k in range(NBLK):
                            j = s_loc * NBLK + k
                            nc.tensor.matmul(
                                pa[:], gt[:, j * F:(j + 1) * F],
                                oh[:, k * 128:(k + 1) * 128],
                                start=(k == 0), stop=(k == NBLK - 1))
                        aggT = wpool.tile([F, 128], F32, tag="aggT")
                        nc.vector.tensor_copy(aggT[:], pa[:])
                        # conv: pre[fo, n] = sum_fi W[fi, fo] * aggT[fi, n]
                        pb = pb_pool.tile([F, 128], F32, tag="pbt", name="pb")
                        nc.tensor.matmul(pb[:], w_sb[:], aggT[:])
                        t1 = wpool.tile([F, 128], F32, tag="t1")
                        nc.vector.tensor_copy(t1[:], pb[:])
                        pt = pt_pool.tile([128, F], F32, tag="pbt", name="pt")
                        nc.tensor.transpose(pt[:], t1[:], ident_sb[:])
                        # epilogue: *rs_in, +bias, (relu * rs_out)
                        t2 = wpool.tile([128, F], F32, tag="t2")
                        nc.vector.tensor_scalar(
                            t2[:], pt[:], rs_in_sb[:, s:s + 1], None,
                            op0=ALU.mult)
                        if layer == 0:
                            t3 = wpool.tile([128, F], F32, tag="t3")
                            nc.vector.tensor_tensor(t3[:], t2[:], b_sb[:],
                                                    op=ALU.add)
                            nc.scalar.activation(
                                stage[:, s_loc * F:(s_loc + 1) * F], t3[:],
                                AF.Relu, scale=rs_out_sb[:, s:s + 1])
                        else:
                            nc.vector.tensor_tensor(
                                h2_sb[:, s * F:(s + 1) * F], t2[:], b_sb[:],
                                op=ALU.add)
                            if INTERLEAVE_TAIL:
                                tail_chunk(s)
                    if layer == 0:
                        dst_ap = h1_d.ap().rearrange("(s p) f -> p s f", p=128)
                        nc.sync.dma_start(
                            dst_ap[:, g * GSL:(g + 1) * GSL, :],
                            stage[:].rearrange("p (s f) -> p s f", f=F))
            if USE_BARRIERS and layer == 0:
                tc.strict_bb_all_engine_barrier()

        # ---- tail: P, S, BN sums (separate phase when not interleaved)
        if True:
            tctx = ctx
            if not INTERLEAVE_TAIL:
                for c in range(NS):
                    tail_chunk(c)

            if debug:
                nc.sync.dma_start(h2_d.ap(), h2_sb[:])
            out_sb = lwpool.tile([1, 2 * CF + 128], F32, tag="outsb")
            # layout: [P(CF) | bn(2F)] from pP splits, then S(CF)
            for i, (o, w) in enumerate(PSPL):
                dst = o
                nc.vector.tensor_copy(out_sb[:, dst:dst + w], pP[i][:])
            for i, (o, w) in enumerate(SSPL):
                nc.vector.tensor_copy(
                    out_sb[:, CF + 2 * F + o:CF + 2 * F + o + w], pS[i][:])
            nc.sync.dma_start(out_d.ap(), out_sb[:])

    nc.compile()
    return nc


_PROGRAM_CACHE = {}


def _get_program(key):
    if key not in _PROGRAM_CACHE:
        _PROGRAM_CACHE[key] = _build_program(*key)
    return _PROGRAM_CACHE[key]


def gcn_forward(x, edge_src, edge_dst, W1, b1, W2, b2, bn_gamma, bn_beta,
                lin_W, lin_b, gsl=None):
    """Full forward pass. x [B, N, F]; returns [B, C]."""
    x = np.asarray(x, np.float32)
    edge_src = np.asarray(edge_src)
    edge_dst = np.asarray(edge_dst)
    W1 = np.asarray(W1, np.float32)
    b1 = np.asarray(b1, np.float32)
    W2 = np.asarray(W2, np.float32)
    b2 = np.asarray(b2, np.float32)
    bn_gamma = np.asarray(bn_gamma, np.float32)
    bn_beta = np.asarray(bn_beta, np.float32)
    lin_W = np.asarray(lin_W, np.float32)
    lin_b = np.asarray(lin_b, np.float32)

    B, N, F = x.shape
    C = lin_W.shape[0]
    NS = N // 128
    n_cores = B

    # padded edges per slice (shared across cores -> same program)
    max_cnt = 1
    for b in range(B):
        cnt = np.bincount(edge_src[b] >> 7, minlength=NS)  # dummy init
        cnt = np.bincount(edge_dst[b] >> 7, minlength=NS)
        max_cnt = max(max_cnt, int(cnt.max()))
    EPS = ((max_cnt + 127) // 128) * 128
    if gsl is None:
        # dma_gather call is validated up to 9216 indices
        gsl = max(1, min(8, NS, 9216 // EPS))
        while NS % gsl:
            gsl -= 1

    nc = _get_program((N, F, EPS, C, n_cores, gsl))

    iota = np.tile(np.arange(128, dtype=np.float32), (128, 1))
    ident = np.eye(F, dtype=np.float32)
    ones = np.ones((128, 1), np.float32)
    b1b = np.tile(b1, (128, 1))
    b2b = np.tile(b2, (128, 1))

    in_maps = []
    for b in range(B):
        idx16, dstloc, rs_out_t, rs_in_t = _prep_graph(
            edge_src[b].astype(np.int64), edge_dst[b].astype(np.int64), N, EPS)
        in_maps.append({
            "x": np.ascontiguousarray(x[b]),
            "idx": idx16,
            "dstloc": dstloc,
            "rs_out": rs_out_t,
            "rs_in": rs_in_t,
            "iota": iota,
            "ident": ident,
            "W1": W1, "W2": W2, "b1b": b1b, "b2b": b2b,
            "ones": ones,
            "lin_W": lin_W,
        })

    res = run_bass_kernel_spmd(nc, in_maps, core_ids=list(range(n_cores)))

    CF = C * F
    P = np.zeros((B, C, F), np.float64)
    s1 = np.zeros(F, np.float64)
    s2 = np.zeros(F, np.float64)
    S = None
    for b in range(B):
        o = res.results[b]["out"][0]
        P[b] = o[:CF].reshape(C, F)
        s1 += o[CF:CF + F]
        s2 += o[CF + F:CF + 2 * F]
        if S is None:
            S = o[CF + 2 * F:2 * CF + 2 * F].reshape(C, F).astype(np.float64)

    cnt = B * N
    mean = s1 / cnt
    var = s2 / cnt - mean * mean
    a = bn_gamma / np.sqrt(var + BN_EPS)
    d = bn_beta - mean * a
    out = (P * a[None, None, :]).sum(-1) + (S * d[None, :]).sum(-1)[None, :] \
        + lin_b[None, :]
    return out.astype(np.float32)


def kernel(**inputs):
    return gcn_forward(
        inputs["x"], inputs["edge_src"], inputs["edge_dst"],
        inputs["W1"], inputs["b1"], inputs["W2"], inputs["b2"],
        inputs["bn_gamma"], inputs["bn_beta"], inputs["lin_W"], inputs["lin_b"])
